# revision 24
# baseline (speedup 1.0000x reference)
"""Trainium2 Bass kernel for fused linear + cross-entropy loss (CCE-style).

Problem: x [4096, 1024] f32, W [50304, 1024] f32, y [4096] int ->
scalar f32 loss = mean over valid tokens of (logsumexp(x @ W.T) - logit[y]).

Strategy (8 NeuronCores, SPMD):
  - Tensor-parallel over vocab: core c owns W rows [c*6288, (c+1)*6288).
    Each core computes sum_v exp(logits[t, v]) over its vocab shard for ALL
    4096 tokens (PE matmul in fp8-e4m3 DoubleRow, ACT exp with fused
    per-partition accumulate).  No max-subtraction is needed: logits have
    std ~0.64 (W scale 0.02), so exp never overflows fp32.
  - fp8 scaling: W is pre-scaled by 64 and x by 16 host-side (keeps values
    out of e4m3 subnormals); the ACT exp applies scale=1/1024 for free.
  - The vocab shard tiles as 12x512 + 144 PE tiles (ragged last tile).
  - Data-parallel over tokens for the target logit: core c computes
    tgt[t] = x[t] . W[y[t]] for its 512 tokens in bf16 on the DVE
    (target rows of W are gathered host-side).
  - Host combines: lse = log(sum over cores of partial sumexp - n_pad),
    loss = mean(lse - tgt) over valid tokens.
"""

import sys

import numpy as np

for _p in ("/opt/trn_rl_repo", "/root/.axon_site/_ro/trn_rl_repo"):
    if _p not in sys.path:
        sys.path.append(_p)

import ml_dtypes

import concourse.bass as bass
import concourse.tile as tile
from concourse import mybir
from concourse.bass_utils import run_bass_kernel_spmd

BF16 = ml_dtypes.bfloat16
FP8 = ml_dtypes.float8_e4m3

V = 50304
H = 1024
N = 4096
NCORES = 8
IGNORE_INDEX = -100

MM_MODE = "fp8"            # "fp8" (DoubleRow) or "bf16"
W_SCALE = 64.0
X_SCALE = 16.0

VSH = V // NCORES          # 6288 vocab rows per core
P = 128
NT = N // P                # 32 token tiles
TSH = N // NCORES          # 512 tokens per core for the target-logit path
TT = TSH // P              # 4 token tiles in the target path

if MM_MODE == "fp8":
    VT = 512               # columns per vocab tile (one PSUM bank)
    NV = 13                # vocab tiles per core; last tile is 144 wide
    KT = H // (2 * P)      # 4 double-row contraction tiles
    GROUPS = [4, 4, 4, 1]  # vocab tiles per PSUM group
else:
    VT = 393
    NV = 16
    KT = H // P            # 8 contraction tiles
    GROUPS = [4, 4, 4, 4]
NG = len(GROUPS)
V_WIDTHS = [min(VT, VSH - v * VT) for v in range(NV)]
XC = 4                     # DMA column chunks for W/x staging


def _patch_tile_drain():
    """Split the TileContext exit drain's sem waits into single-wait
    instructions: this walrus build rejects >1 sync wait per instruction."""
    import bass_rust
    from concourse.vector_clock import ScopedClock

    if getattr(tile.TileContext, "_drain_patched", False):
        return

    def _drain_and_barrier(self, tick_clock, wait_clock):
        nc = self.nc
        probe = nc.sync.drain()
        wait_clock.add_sem_waits(
            probe.ins, ScopedClock({None: tick_clock.global_clock})
        )
        si = probe.ins.sync_info
        waits = list(si.on_wait) if si and si.on_wait else []
        if len(waits) > 1:
            probe.ins.sync_info.on_wait = []
            for w in waits:
                h = bass_rust.SemaphoreHandle(name=w.ant_name, num=w.id)
                nc.sync.wait_ge(h, w.wait_value)
            nc.sync.drain()
        nc.all_engine_barrier()
        popped = nc._tile_sem_poison_stack.pop()
        assert popped is self._sem_poison
        nc.clear_and_free_semaphores(list(self.sems.allocated().values()))
        nc.all_engine_barrier()

    tile.TileContext._drain_and_barrier = _drain_and_barrier
    tile.TileContext._drain_patched = True


def _split_sync_waits(nc, limit=1):
    """Hoist excess sync waits onto single-wait EventSemaphore instructions
    inserted just before the offender on the same engine queue (engines
    drain their queue in order, so the semantics are identical)."""
    import bass_rust

    def make_wait_inst(engine, w):
        ev = bass_rust.InstEventSemaphore(name=nc.get_next_instruction_name())
        ev.engine = engine
        h = bass_rust.SemaphoreHandle(name=w.ant_name, num=w.id)
        bass_rust.wait_op(ev, h, w.wait_value, "sem-ge", False)
        nc.register_instruction(ev, overwrite=True)
        return ev

    n_new = 0
    for bb in nc.m.functions[0].blocks:
        insts = bb.instructions
        out = []
        changed = False
        for inst in insts:
            si = inst.sync_info
            waits = list(si.on_wait) if si and si.on_wait else []
            movable = [
                w for w in waits
                if w.wait_reg is None and w.wait_mode == "sem-ge-imm"
            ]
            if len(waits) > limit and movable:
                n_move = min(len(waits) - limit, len(movable))
                movable = movable[:n_move]
                keep = [w for w in waits if w not in movable]
                for w in movable:
                    out.append(make_wait_inst(inst.engine, w))
                    n_new += 1
                inst.sync_info.on_wait = keep
                changed = True
            out.append(inst)
        if changed:
            bb.instructions = out
    return n_new


def build_bass():
    """Build the single-core Bass program (SPMD: same program, per-core data)."""
    _patch_tile_drain()
    nc = bass.Bass(trn_type="TRN2")

    bf = mybir.dt.bfloat16
    f32 = mybir.dt.float32
    fp8 = mybir.dt.float8e4
    mm_dt = fp8 if MM_MODE == "fp8" else bf
    KR = 2 if MM_MODE == "fp8" else 1      # contraction rows per k-tile / P
    perf_mode = (
        mybir.MatmulPerfMode.DoubleRow if MM_MODE == "fp8" else None
    )
    inv_scale = (
        1.0 / (W_SCALE * X_SCALE) if MM_MODE == "fp8" else 1.0
    )

    # Inputs: [KT, P, KR, cols] flattened to [KT*P, KR*cols] row-major so the
    # per-k-tile DMA is one contiguous block.
    xT = nc.dram_tensor("xT", [KT * P, KR * N], mm_dt, kind="ExternalInput")
    wT = nc.dram_tensor("wT", [KT * P, KR * VSH], mm_dt, kind="ExternalInput")
    xc = nc.dram_tensor("xc", [P, TT * H], bf, kind="ExternalInput")
    wy = nc.dram_tensor("wy", [P, TT * H], bf, kind="ExternalInput")
    sumexp_out = nc.dram_tensor("sumexp_out", [P, NT * NG], f32, kind="ExternalOutput")
    tgt_out = nc.dram_tensor("tgt_out", [P, TT], f32, kind="ExternalOutput")

    with tile.TileContext(nc) as tc:
        with (
            tc.tile_pool(name="wpool", bufs=1) as wpool,
            tc.tile_pool(name="xpool", bufs=1) as xpool,
            tc.tile_pool(name="iopool", bufs=1) as iopool,
            tc.tile_pool(name="scratch", bufs=2) as spool,
            tc.tile_pool(name="psum", bufs=2, space="PSUM") as psum,
        ):
            w_sb = [
                wpool.tile([P, KR, VSH], mm_dt, name=f"w_sb{k}", tag=f"w{k}")
                for k in range(KT)
            ]
            x_sb = [
                xpool.tile([P, KR, N], mm_dt, name=f"x_sb{k}", tag=f"x{k}")
                for k in range(KT)
            ]
            # Stage loads in chunks aligned to the PSUM-group column ranges so
            # each matmul pass only waits for its own columns.  Order: x
            # first (every pass needs it), then the ragged W tail (the first,
            # cheapest pass), then the full-width group chunks.
            gw = max(GROUPS) * VT
            wedges = [0] + [min((i + 1) * gw, VSH) for i in range(-(-VSH // gw))]
            wranges = [(wedges[i], wedges[i + 1]) for i in range(len(wedges) - 1)]
            xchunk = N // XC
            wT_r = wT.rearrange("(k p) (r v) -> k p r v", k=KT, r=KR)
            xT_r = xT.rearrange("(k p) (r n) -> k p r n", k=KT, r=KR)
            for k in range(KT):
                nc.sync.dma_start(
                    x_sb[k][:, :, 0:xchunk], xT_r[k, :, :, 0:xchunk]
                )
            for c, (c0, c1) in enumerate(wranges):
                for k in range(KT):
                    nc.sync.dma_start(
                        w_sb[k][:, :, c0:c1], wT_r[k, :, :, c0:c1]
                    )
                if c + 1 < XC:
                    s0, s1 = (c + 1) * xchunk, (c + 2) * xchunk
                    for k in range(KT):
                        nc.sync.dma_start(
                            x_sb[k][:, :, s0:s1], xT_r[k, :, :, s0:s1]
                        )

            xc_sb = iopool.tile([P, TT * H], bf, name="xc_sb")
            wy_sb = iopool.tile([P, TT * H], bf, name="wy_sb")
            nc.sync.dma_start(xc_sb[:], xc[:, :])
            nc.sync.dma_start(wy_sb[:], wy[:, :])

            sums_sb = iopool.tile([P, NT * NG], f32, name="sums_sb")
            tgt_sb = iopool.tile([P, TT], f32, name="tgt_sb")

            # Target-logit path: tgt[p, j] = sum_h xc[p, j*H + h] * wy[p, j*H + h]
            for j in range(TT):
                prod_sb = spool.tile([P, H], bf, name="prod_sb", tag="prod")
                nc.vector.tensor_tensor(
                    prod_sb[:],
                    xc_sb[:, j * H : (j + 1) * H],
                    wy_sb[:, j * H : (j + 1) * H],
                    mybir.AluOpType.mult,
                )
                nc.vector.tensor_reduce(
                    tgt_sb[:, j : j + 1],
                    prod_sb[:],
                    mybir.AxisListType.X,
                    mybir.AluOpType.add,
                )

            # Main path: logits tile [128 tokens, VG x VT vocab] accumulated
            # over k in PSUM (VG separate banks), then one fused exp+row-sum
            # ACT instruction per group via a strided 3-D AP.
            #
            # Uniform full-width groups only: the ragged last vocab tile is
            # deferred to a separate phase so the PE<->ACT ping-pong over the
            # two PSUM slots never pairs a short ACT with a full PE refill.
            def emit_group(t, vg, vlist, widths):
                ptile = psum.tile([P, max_vg, VT], f32, name="ps", tag="ps")
                for k in range(KT):
                    for i, v in enumerate(vlist):
                        w_i = widths[i]
                        if KR == 2:
                            lhsT = x_sb[k][:, :, t * P : (t + 1) * P]
                            rhs = w_sb[k][:, :, v * VT : v * VT + w_i]
                        else:
                            lhsT = x_sb[k][:, 0, t * P : (t + 1) * P]
                            rhs = w_sb[k][:, 0, v * VT : v * VT + w_i]
                        nc.tensor.matmul(
                            ptile[:, i, :w_i],
                            lhsT=lhsT,
                            rhs=rhs,
                            start=(k == 0),
                            stop=(k == KT - 1),
                            perf_mode=perf_mode,
                        )
                vw = widths[0]
                assert all(w == vw for w in widths)
                nc.scalar.activation(
                    ptile[:, : len(vlist), :vw],
                    ptile[:, : len(vlist), :vw],
                    mybir.ActivationFunctionType.Exp,
                    scale=inv_scale,
                    accum_out=sums_sb[:, t * NG + vg : t * NG + vg + 1],
                )

            max_vg = max(GROUPS)
            n_full = NV if V_WIDTHS[-1] == VT else NV - 1
            full_groups = []
            v0 = 0
            for vgn in GROUPS:
                vlist = [v for v in range(v0, min(v0 + vgn, n_full))]
                if vlist:
                    full_groups.append(vlist)
                v0 += vgn
            # Group-major order: one full pass over t per vocab group, so
            # each pass only touches its own slice of W and the W DMA stream
            # stays ahead of the PE.  The cheap ragged pass (144 cols of W)
            # runs first, covering the DMA ramp-up.
            for vg, vlist in enumerate(full_groups):
                for t in range(NT):
                    emit_group(t, vg, vlist, [VT] * len(vlist))
            if n_full < NV:
                # Ragged tail (last vocab tile, V_WIDTHS[-1] cols): batch
                # max_vg token-tiles into one PSUM slot (one bank each), a
                # single exp ACT over all banks (no accum -- the banks hold
                # different tokens), then idle-DVE row-sums per bank.
                wtl = V_WIDTHS[-1]
                for tb in range(NT // max_vg):
                    ptile = psum.tile([P, max_vg, VT], f32, name="ps", tag="ps")
                    for k in range(KT):
                        for j in range(max_vg):
                            t = tb * max_vg + j
                            if KR == 2:
                                lhsT = x_sb[k][:, :, t * P : (t + 1) * P]
                                rhs = w_sb[k][:, :, n_full * VT : n_full * VT + wtl]
                            else:
                                lhsT = x_sb[k][:, 0, t * P : (t + 1) * P]
                                rhs = w_sb[k][:, 0, n_full * VT : n_full * VT + wtl]
                            nc.tensor.matmul(
                                ptile[:, j, :wtl],
                                lhsT=lhsT,
                                rhs=rhs,
                                start=(k == 0),
                                stop=(k == KT - 1),
                                perf_mode=perf_mode,
                            )
                    nc.scalar.activation(
                        ptile[:, :, :wtl],
                        ptile[:, :, :wtl],
                        mybir.ActivationFunctionType.Exp,
                        scale=inv_scale,
                    )
                    for j in range(max_vg):
                        t = tb * max_vg + j
                        col = t * NG + len(full_groups)
                        nc.vector.tensor_reduce(
                            sums_sb[:, col : col + 1],
                            ptile[:, j, :wtl],
                            mybir.AxisListType.X,
                            mybir.AluOpType.add,
                        )

            nc.sync.dma_start(sumexp_out[:, :], sums_sb[:])
            nc.sync.dma_start(tgt_out[:, :], tgt_sb[:])

    _split_sync_waits(nc)
    return nc


def prepare_inputs(x, W, y):
    """Host-side sharding: cast/scale, pack DoubleRow layout, gather target
    rows."""
    x = np.asarray(x)
    W = np.asarray(W)
    y = np.asarray(y)

    KR = 2 if MM_MODE == "fp8" else 1

    if MM_MODE == "fp8":
        x_mm = (x * X_SCALE).astype(FP8)            # [N, H]
        W_mm = (W * W_SCALE).astype(FP8)            # [V, H]
    else:
        x_mm = x.astype(BF16)
        W_mm = W.astype(BF16)

    # [cols, H] -> transposed+packed [KT*P, KR*cols]:
    # element (h, c) lands at row (h // (KR*P))*P + (h % P),
    # col ((h // P) % KR)*cols + c
    def pack(mat):                                   # mat [C, H] -> [KT*P, KR*C]
        C = mat.shape[0]
        mT = np.ascontiguousarray(mat.T)             # [H, C]
        m4 = mT.reshape(KT, KR, P, C)                # h = k*KR*P + r*P + p
        m4 = m4.transpose(0, 2, 1, 3)                # [KT, P, KR, C]
        return np.ascontiguousarray(m4.reshape(KT * P, KR * C))

    xT_packed = pack(x_mm)

    x_bf = x.astype(BF16)
    y_idx = np.clip(y, 0, V - 1).astype(np.int64)
    Wy = W[y_idx].astype(BF16)                       # [N, H]

    in_maps = []
    for c in range(NCORES):
        W_c = W_mm[c * VSH : (c + 1) * VSH]
        xc_c = (
            x_bf[c * TSH : (c + 1) * TSH]
            .reshape(TT, P, H)
            .transpose(1, 0, 2)
            .reshape(P, TT * H)
        )
        wy_c = (
            Wy[c * TSH : (c + 1) * TSH]
            .reshape(TT, P, H)
            .transpose(1, 0, 2)
            .reshape(P, TT * H)
        )
        in_maps.append(
            {
                "xT": xT_packed,
                "wT": pack(W_c),
                "xc": np.ascontiguousarray(xc_c),
                "wy": np.ascontiguousarray(wy_c),
            }
        )
    return in_maps


def combine_outputs(results, y):
    """Host-side unshard: combine per-core partial sumexp and target logits."""
    y = np.asarray(y)
    total_sumexp = np.zeros(N, dtype=np.float64)
    tgt = np.zeros(N, dtype=np.float64)
    for c in range(NCORES):
        s = np.asarray(results[c]["sumexp_out"], dtype=np.float64)  # [P, NT*NG]
        s = s.reshape(P, NT, NG).sum(axis=2)                        # [P, NT]
        total_sumexp += s.T.reshape(N)                              # token = t*P + p
        tg = np.asarray(results[c]["tgt_out"], dtype=np.float64)    # [P, TT]
        tgt[c * TSH : (c + 1) * TSH] = tg.T.reshape(TSH)            # token = j*P + p

    lse = np.log(total_sumexp)
    valid = y != IGNORE_INDEX
    count = max(int(valid.sum()), 1)
    loss = np.where(valid, lse - tgt, 0.0).sum() / count
    return np.float32(loss)


_BASS_CACHE = {}


def get_nc():
    if "nc" not in _BASS_CACHE:
        _BASS_CACHE["nc"] = build_bass()
    return _BASS_CACHE["nc"]


def kernel(x, W, y):
    nc = get_nc()
    in_maps = prepare_inputs(x, W, y)
    res = run_bass_kernel_spmd(nc, in_maps, core_ids=list(range(NCORES)))
    return combine_outputs(res.results, y)
